# revision 2
# baseline (speedup 1.0000x reference)
"""Trainium2 Bass kernel for nn_AggrSum (segment_sum of H rows by X_node).

out[v, :] = sum_{n : X_node[n] == v} H[n, :],  H [1600000, 128] f32,
X_node [1600000] int64 in [0, 100000).

Strategy (8 NeuronCores, SPMD single program):
  * Host planning: argsort X_node; the V axis is tiled into 128-segment
    windows. Windows are ranked by row count and dealt round-robin to
    (core, slot) so that the 8 windows sharing a slot need the same
    chunk count -> a single SPMD program with per-slot chunk counts and
    almost no padding. Each core's rows are packed host-side into its
    input tensor in the exact SBUF layout the kernel consumes
    ([128 partitions, total_chunks, 128]); H is cast to fp16 (256 B per
    row -> half the fp32 HBM traffic; quantization rel-err ~1e-4 vs the
    2e-2 gate).
  * Device, per window slot: one contiguous DMA streams the slot's rows
    (window-group batched); a one-hot selection matrix
    onehot[node, seg] = (xrel[node] == seg) is built from a resident
    iota tile with one is_equal tensor_tensor on DVE; per 128-row chunk
    ONE matmul (lhsT=onehot chunk, rhs=[128, 128] fp16 rows)
    accumulates PSUM [128 segs, 128]; an ACT copy moves PSUM to SBUF
    and the result is written out.
  * Host scatters the per-core window blocks back to V order.

Segment-sharded output means no cross-core reduction is needed; each
core streams 1/8 of the rows once (~51 MB) and writes 6.4 MB.
"""
import dataclasses

import numpy as np

import concourse.bass as bass
import concourse.mybir as mybir
import concourse.tile as tile
from concourse import bacc
from concourse import bass_utils

P = 128
D = 128
N_CORES = 8
F32 = mybir.dt.float32
F16 = mybir.dt.float16

_CACHE = {}


def _plan_schedule(X, n_cores):
    N = X.shape[0]
    V = int(X.max()) + 1 if N else 1
    perm = np.argsort(X)
    Xs = X[perm].astype(np.int64)

    n_windows_total = -(-V // P)
    W = -(-n_windows_total // n_cores)
    NW = W * n_cores

    win_of_node = Xs // P
    counts = np.bincount(win_of_node, minlength=NW)[:NW]
    starts = np.zeros(NW + 1, dtype=np.int64)
    np.cumsum(counts, out=starts[1:])

    wsorted = np.argsort(-counts, kind="stable")
    assign = wsorted.reshape(W, n_cores)
    Ks = np.maximum(1, -(-counts[assign].max(axis=1) // P)).astype(np.int64)
    off = np.zeros(W + 1, dtype=np.int64)
    np.cumsum(Ks, out=off[1:])
    TOT = int(off[-1])

    order = np.full((n_cores, TOT * P), -1, dtype=np.int64)
    xrel = np.full((n_cores, P, TOT), -1.0, dtype=np.float32)
    for c in range(n_cores):
        ov = order[c].reshape(TOT, P)
        xr = xrel[c]
        for s in range(W):
            gwi = int(assign[s, c])
            a, b = int(starts[gwi]), int(starts[gwi + 1])
            cnt = b - a
            if cnt == 0:
                continue
            t0 = int(off[s])
            nch = (cnt + P - 1) // P
            ov[t0:t0 + nch].ravel()[:cnt] = perm[a:b]
            tmp = np.full(nch * P, -1.0, np.float32)
            tmp[:cnt] = (Xs[a:b] - gwi * P).astype(np.float32)
            xr[:, t0:t0 + nch] = tmp.reshape(-1, P).T

    iota = np.ascontiguousarray(np.broadcast_to(
        np.arange(P, dtype=np.float16)[None, :], (P, P)))

    return dict(
        V=V, W=W, Ks=Ks, off=off, TOT=TOT, n_cores=n_cores,
        assign=assign, order=order,
        xrel=xrel.astype(np.float16),
        iota=iota,
    )


def _make_in_maps(H, meta):
    n_cores, TOT = meta["n_cores"], meta["TOT"]
    H_hi = H.astype(np.float16)
    maps = []
    for c in range(n_cores):
        flat = meta["order"][c]
        sel = np.clip(flat, 0, None)
        hi = H_hi[sel]
        hi[flat < 0] = 0
        h = hi.reshape(TOT, P, D)
        h = np.ascontiguousarray(h.transpose(1, 0, 2))
        maps.append({
            "h": h,
            "xrel": meta["xrel"][c],
            "iota": meta["iota"],
        })
    return maps


def _assemble_output(res_outs, meta):
    n_cores, W, V = meta["n_cores"], meta["W"], meta["V"]
    full = np.zeros((W * n_cores * P, D), dtype=np.float32)
    assign = meta["assign"]
    for c in range(n_cores):
        oc = res_outs[c].reshape(W, P, D)
        for s in range(W):
            gwi = int(assign[s, c])
            full[gwi * P:(gwi + 1) * P] = oc[s]
    return full[:V]


def _bcast_mid(ap, k, block, mode):
    part = ap.ap[0]
    if mode == "rep_block":
        assert ap.ap[1][1] == block, ap.ap
        new = [part, [0, k], [ap.ap[1][0], block]]
    else:
        assert ap.ap[1][1] == k, ap.ap
        new = [part, [ap.ap[1][0], k], [0, block]]
    return dataclasses.replace(ap, ap=new)


def _build_nc(W, Ks, off, n_cores, wins_per_load=4, nbufs=8,
              out_on_scalar=True):
    Ks = [int(k) for k in Ks]
    off = [int(o) for o in off]
    TOT = off[-1]
    # keep the gather pool within the SBUF budget
    gt_kb = max(off[min(w + wins_per_load, W)] - off[w]
                for w in range(0, W, wins_per_load)) * D * 2 / 1024.0
    resident_kb = TOT * 2 / 1024.0 + P * 2 / 1024.0
    nbufs = max(2, min(nbufs, int((175 - resident_kb - 16) // gt_kb)))
    nc = bacc.Bacc("TRN2", target_bir_lowering=False, debug=False,
                   num_devices=n_cores)
    h = nc.dram_tensor("h", [P, TOT, D], F16, kind="ExternalInput").ap()
    xrel_d = nc.dram_tensor("xrel", [P, TOT], F16, kind="ExternalInput").ap()
    iota_d = nc.dram_tensor("iota", [P, P], F16, kind="ExternalInput").ap()
    out_d = nc.dram_tensor("out", [W * P, D], F32, kind="ExternalOutput").ap()

    with tile.TileContext(nc) as tc:
        with (
            tc.tile_pool(name="res", bufs=1) as res,
            tc.tile_pool(name="gat", bufs=nbufs) as gat,
            tc.tile_pool(name="oh", bufs=4) as ohp,
            tc.tile_pool(name="ps", bufs=4, space="PSUM") as ps,
            tc.tile_pool(name="osb", bufs=4) as osb,
        ):
            xrel_sb = res.tile([P, TOT], F16)
            iota_sb = res.tile([P, P], F16)
            nc.sync.dma_start(out=xrel_sb[:], in_=xrel_d[:])
            nc.sync.dma_start(out=iota_sb[:], in_=iota_d[:])

            gt = None
            gt_base = 0
            for w in range(W):
                K = Ks[w]
                t0 = off[w]
                if w % wins_per_load == 0:
                    t1 = off[min(w + wins_per_load, W)]
                    gt = gat.tile([P, (t1 - t0) * D], F16, tag="gt")
                    nc.sync.dma_start(
                        out=gt[:],
                        in_=h[:, t0:t1, :].rearrange("p t d -> p (t d)"))
                    gt_base = t0
                rel = t0 - gt_base

                oh = ohp.tile([P, K * D], F16, tag="oh")
                nc.vector.tensor_tensor(
                    out=oh[:],
                    in0=_bcast_mid(iota_sb[:, :P], K, P, "rep_block"),
                    in1=_bcast_mid(xrel_sb[:, t0:t0 + K], K, P, "rep_elem"),
                    op=mybir.AluOpType.is_equal,
                )
                pt = ps.tile([P, D], F32, tag="pt")
                for j in range(K):
                    nc.tensor.matmul(
                        out=pt[:],
                        lhsT=oh[:, j * D:(j + 1) * D],
                        rhs=gt[:, (rel + j) * D:(rel + j + 1) * D],
                        start=(j == 0), stop=(j == K - 1),
                    )
                ot = osb.tile([P, D], F32, tag="ot")
                nc.scalar.copy(out=ot[:], in_=pt[:])
                oeng = nc.scalar if out_on_scalar else nc.sync
                oeng.dma_start(out=out_d[w * P:(w + 1) * P, :], in_=ot[:])

    nc.compile()
    return nc


def prepare(H, X_node):
    """Plan + build + shard. Returns (nc, in_maps, meta). Cached on the
    schedule signature so repeated kernel() calls reuse the compiled
    program."""
    H = np.ascontiguousarray(np.asarray(H, dtype=np.float32))
    X = np.asarray(X_node).astype(np.int64)
    assert H.ndim == 2 and H.shape[1] == D and X.shape == (H.shape[0],)

    meta = _plan_schedule(X, N_CORES)
    key = (meta["W"], tuple(int(k) for k in meta["Ks"]))
    if key not in _CACHE:
        _CACHE[key] = _build_nc(meta["W"], meta["Ks"], meta["off"], N_CORES)
    nc = _CACHE[key]
    in_maps = _make_in_maps(H, meta)
    return nc, in_maps, meta


def kernel(H, X_node):
    nc, in_maps, meta = prepare(H, X_node)
    res = bass_utils.run_bass_kernel_spmd(
        nc, in_maps, core_ids=list(range(N_CORES)))
    out = _assemble_output([res.results[c]["out"] for c in range(N_CORES)],
                           meta)
    return out.astype(np.float32)
